# revision 1
# baseline (speedup 1.0000x reference)
"""Trainium2 Bass kernel for nn_ContextualViewModel (gnn_message_passing).

Reference semantics:
    sx, sy = station_ids // 512, station_ids % 512
    s = sum_k x[sx_k, sy_k] @ W          # a single (128,) vector
    out = broadcast_to(s, (512, 512, 128))

The compute is tiny; the problem is memory-bound on writing the 128 MiB
output. Sharding: split the (i,j) grid of the output across 8 cores
(64 rows of 512 each -> 16 MiB per core). The K=128 gathered station rows
and W are replicated to every core (gathered host-side while slicing
inputs, per the sharding hint). Each core computes s with two PE matmuls,
replicates it into a wide SBUF tile, and streams its output shard to HBM.
"""

import sys

import numpy as np

try:
    import concourse  # noqa: F401
except ImportError:  # pragma: no cover
    sys.path.insert(0, "/opt/trn_rl_repo")

H, WD, K = 512, 512, 128
N_CORES = 8
ROWS_PER_CORE = H // N_CORES          # 64 rows of the (i) axis per core
SHARD_FLOATS = ROWS_PER_CORE * WD * K  # 4,194,304 floats = 16 MiB

# Output shard is viewed as [N_CHUNKS, 128, CHUNK_F] for the store DMAs:
# a [128, CHUNK_F] SBUF tile holding s replicated is written N_CHUNKS times.
CHUNK_F = 4096                         # floats per partition per store DMA
CHUNK_FLOATS = 128 * CHUNK_F           # 2 MiB per DMA
N_CHUNKS = SHARD_FLOATS // CHUNK_FLOATS  # 8

_NC = None


def _build():
    import concourse.bacc as bacc
    import concourse.mybir as mybir
    import concourse.tile as tile

    f32 = mybir.dt.float32

    nc = bacc.Bacc(
        "TRN2", target_bir_lowering=False, debug=False, num_devices=N_CORES
    )

    g_dram = nc.dram_tensor("g", [K, K], f32, kind="ExternalInput")
    w_dram = nc.dram_tensor("w", [K, K], f32, kind="ExternalInput")
    out_dram = nc.dram_tensor(
        "out", [N_CHUNKS, 128, CHUNK_F], f32, kind="ExternalOutput"
    )

    with tile.TileContext(nc) as tc:
        with (
            tc.tile_pool(name="sbuf", bufs=1) as pool,
            tc.tile_pool(name="psum", bufs=1, space="PSUM") as psum,
        ):
            gt = pool.tile([K, K], f32)
            wt = pool.tile([K, K], f32)
            nc.sync.dma_start(gt[:], g_dram[:])
            nc.sync.dma_start(wt[:], w_dram[:])

            ones_col = pool.tile([K, 1], f32)
            nc.vector.memset(ones_col[:], 1.0)
            ones_row = pool.tile([1, K], f32)
            nc.vector.memset(ones_row[:], 1.0)

            # u[c] = sum_k g[k, c]   (contract over the k partitions)
            u_ps = psum.tile([K, 1], f32)
            nc.tensor.matmul(u_ps[:], gt[:], ones_col[:], start=True, stop=True)
            u_sb = pool.tile([K, 1], f32)
            nc.vector.tensor_copy(u_sb[:], u_ps[:])

            # s[d] = sum_c u[c] * W[c, d]
            s_ps = psum.tile([1, K], f32)
            nc.tensor.matmul(s_ps[:], u_sb[:], wt[:], start=True, stop=True)
            s_sb = pool.tile([1, K], f32)
            nc.vector.tensor_copy(s_sb[:], s_ps[:])

            # outer product ones(128,1) @ s(1,128): every partition = s
            b_ps = psum.tile([128, K], f32)
            nc.tensor.matmul(b_ps[:], ones_row[:], s_sb[:], start=True, stop=True)

            # replicate along the free dim: 128 -> CHUNK_F floats/partition
            rep = pool.tile([128, CHUNK_F], f32)
            nc.vector.tensor_copy(rep[:, 0:K], b_ps[:])
            w_cur = K
            while w_cur < CHUNK_F:
                nc.vector.tensor_copy(rep[:, w_cur : 2 * w_cur], rep[:, 0:w_cur])
                w_cur *= 2

            for c in range(N_CHUNKS):
                nc.sync.dma_start(out_dram[c], rep[:])

    nc.compile()
    return nc


def _get_nc():
    global _NC
    if _NC is None:
        _NC = _build()
    return _NC


def _run(g: np.ndarray, w: np.ndarray, trace: bool = False):
    from concourse.bass_utils import run_bass_kernel_spmd

    nc = _get_nc()
    in_maps = [{"g": g, "w": w} for _ in range(N_CORES)]
    return run_bass_kernel_spmd(nc, in_maps, list(range(N_CORES)), trace=trace)


def kernel(x: np.ndarray, W: np.ndarray, station_ids: np.ndarray) -> np.ndarray:
    x = np.asarray(x, dtype=np.float32)
    W = np.ascontiguousarray(np.asarray(W, dtype=np.float32))
    sid = np.asarray(station_ids).astype(np.int64)

    sx = sid // H
    sy = sid % WD
    g = np.ascontiguousarray(x[sx, sy])  # (K, K) replicated station rows

    res = _run(g, W).results
    shards = [res[c]["out"].reshape(ROWS_PER_CORE, WD, K) for c in range(N_CORES)]
    return np.concatenate(shards, axis=0)


# revision 3
# speedup vs baseline: 1.1211x; 1.1211x over previous
"""Trainium2 Bass kernel for nn_ContextualViewModel (gnn_message_passing).

Reference semantics:
    sx, sy = station_ids // 512, station_ids % 512
    s = sum_k x[sx_k, sy_k] @ W          # a single (128,) vector
    out = broadcast_to(s, (512, 512, 128))

The compute is tiny; the problem is memory-bound on writing the 128 MiB
output. Sharding: split the (i,j) grid of the output across 8 cores
(64 rows of 512 each -> 16 MiB per core). The K=128 gathered station rows
and W are replicated to every core (gathered host-side while slicing
inputs, per the sharding hint). Each core computes s with two PE matmuls,
replicates it into a wide SBUF tile, and streams its output shard to HBM.
"""

import sys

import numpy as np

try:
    import concourse  # noqa: F401
except ImportError:  # pragma: no cover
    sys.path.insert(0, "/opt/trn_rl_repo")

H, WD, K = 512, 512, 128
N_CORES = 8
ROWS_PER_CORE = H // N_CORES          # 64 rows of the (i) axis per core
SHARD_FLOATS = ROWS_PER_CORE * WD * K  # 4,194,304 floats = 16 MiB

# Output shard is viewed as [N_CHUNKS, 128, CHUNK_F] for the store DMAs:
# a [128, CHUNK_F] SBUF tile holding s replicated is written N_CHUNKS times.
CHUNK_F = 1024                         # floats per partition per store DMA
CHUNK_FLOATS = 128 * CHUNK_F           # 0.5 MiB per DMA
N_CHUNKS = SHARD_FLOATS // CHUNK_FLOATS  # 32

_NC = None


def _build():
    import concourse.bacc as bacc
    import concourse.mybir as mybir
    import concourse.tile as tile

    f32 = mybir.dt.float32

    nc = bacc.Bacc(
        "TRN2", target_bir_lowering=False, debug=False, num_devices=N_CORES
    )

    g_dram = nc.dram_tensor("g", [K, K], f32, kind="ExternalInput")
    w_dram = nc.dram_tensor("w", [K, K], f32, kind="ExternalInput")
    out_dram = nc.dram_tensor(
        "out", [N_CHUNKS, 128, CHUNK_F], f32, kind="ExternalOutput"
    )

    with tile.TileContext(nc) as tc:
        with (
            tc.tile_pool(name="sbuf", bufs=1) as pool,
            tc.tile_pool(name="psum", bufs=1, space="PSUM") as psum,
        ):
            gt = pool.tile([K, K], f32)
            wt = pool.tile([K, K], f32)
            nc.sync.dma_start(gt[:], g_dram[:])
            nc.scalar.dma_start(wt[:], w_dram[:])

            ones_col = pool.tile([K, 1], f32)
            nc.vector.memset(ones_col[:], 1.0)
            ones_row = pool.tile([1, K], f32)
            nc.vector.memset(ones_row[:], 1.0)

            # u[c] = sum_k g[k, c]   (contract over the k partitions)
            u_ps = psum.tile([K, 1], f32)
            nc.tensor.matmul(u_ps[:], gt[:], ones_col[:], start=True, stop=True)
            u_sb = pool.tile([K, 1], f32)
            nc.vector.tensor_copy(u_sb[:], u_ps[:])

            # s[d] = sum_c u[c] * W[c, d]
            s_ps = psum.tile([1, K], f32)
            nc.tensor.matmul(s_ps[:], u_sb[:], wt[:], start=True, stop=True)

            # s replicated 4x along the free dim (still one partition)
            s4 = pool.tile([1, 4 * K], f32)
            nc.vector.tensor_copy(s4[:, 0:K], s_ps[:])
            nc.vector.tensor_copy(s4[:, K : 2 * K], s4[:, 0:K])
            nc.vector.tensor_copy(s4[:, 2 * K : 4 * K], s4[:, 0 : 2 * K])

            # outer product ones(128,1) @ s4(1,512): every partition = s x4
            b_ps = psum.tile([128, 4 * K], f32)
            nc.tensor.matmul(b_ps[:], ones_row[:], s4[:], start=True, stop=True)

            # replicate along the free dim: 512 -> CHUNK_F floats/partition
            rep = pool.tile([128, CHUNK_F], f32)
            nc.vector.tensor_copy(rep[:, 0 : 4 * K], b_ps[:])
            w_cur = 4 * K
            while w_cur < CHUNK_F:
                nc.vector.tensor_copy(rep[:, w_cur : 2 * w_cur], rep[:, 0:w_cur])
                w_cur *= 2

            # stream the shard out; alternate the two HWDGE issue engines
            for c in range(N_CHUNKS):
                eng = nc.sync if c % 2 == 0 else nc.scalar
                eng.dma_start(out_dram[c], rep[:])

    nc.compile()
    return nc


def _get_nc():
    global _NC
    if _NC is None:
        _NC = _build()
    return _NC


def _run(g: np.ndarray, w: np.ndarray, trace: bool = False):
    from concourse.bass_utils import run_bass_kernel_spmd

    nc = _get_nc()
    in_maps = [{"g": g, "w": w} for _ in range(N_CORES)]
    return run_bass_kernel_spmd(nc, in_maps, list(range(N_CORES)), trace=trace)


def kernel(x: np.ndarray, W: np.ndarray, station_ids: np.ndarray) -> np.ndarray:
    x = np.asarray(x, dtype=np.float32)
    W = np.ascontiguousarray(np.asarray(W, dtype=np.float32))
    sid = np.asarray(station_ids).astype(np.int64)

    sx = sid // H
    sy = sid % WD
    g = np.ascontiguousarray(x[sx, sy])  # (K, K) replicated station rows

    res = _run(g, W).results
    shards = [res[c]["out"].reshape(ROWS_PER_CORE, WD, K) for c in range(N_CORES)]
    return np.concatenate(shards, axis=0)


# revision 9
# speedup vs baseline: 1.1572x; 1.0322x over previous
"""Trainium2 Bass kernel for nn_ContextualViewModel (gnn_message_passing).

Reference semantics:
    sx, sy = station_ids // 512, station_ids % 512
    s = sum_k x[sx_k, sy_k] @ W          # a single (128,) vector
    out = broadcast_to(s, (512, 512, 128))

The compute is tiny; the problem is memory-bound on writing the 128 MiB
output. Sharding: split the (i,j) grid of the output across 8 cores
(64 rows of 512 each -> 16 MiB per core). The K=128 gathered station rows
and W are replicated to every core (gathered host-side while slicing
inputs, per the sharding hint). Each core computes s with two PE matmuls,
replicates it into a wide SBUF tile, and streams its output shard to HBM.
"""

import sys

import numpy as np

try:
    import concourse  # noqa: F401
except ImportError:  # pragma: no cover
    sys.path.insert(0, "/opt/trn_rl_repo")

H, WD, K = 512, 512, 128
N_CORES = 8
ROWS_PER_CORE = H // N_CORES          # 64 rows of the (i) axis per core
SHARD_FLOATS = ROWS_PER_CORE * WD * K  # 4,194,304 floats = 16 MiB

# Output shard is viewed as [N_CHUNKS, 128, CHUNK_F] for the store DMAs:
# a [128, CHUNK_F] SBUF tile holding s replicated is written N_CHUNKS times.
CHUNK_F = 2048                         # floats per partition per store DMA
CHUNK_FLOATS = 128 * CHUNK_F           # 1 MiB per DMA
N_CHUNKS = SHARD_FLOATS // CHUNK_FLOATS  # 16

_NC = None
USE_RAW = True


def _build_raw():
    """Raw bacc build: manual semaphores, no Tile scheduling/drain overhead.

    Engine plan (per core):
      sync:   load g          -> [wait rep ready] -> store even chunks -> wait all
      scalar: load W          -> [wait rep ready] -> store odd chunks  -> wait all
      tensor: mm1 u=g^T.1, mm2 s=u^T.W, mm3 b=1.s  (gated on loads + DVE copies)
      vector: memset ones, PSUM->SBUF copies, widen rep 128 -> CHUNK_F
    """
    from contextlib import ExitStack

    import concourse.bacc as bacc
    import concourse.mybir as mybir

    f32 = mybir.dt.float32
    nc = bacc.Bacc(
        "TRN2", target_bir_lowering=False, debug=False, num_devices=N_CORES
    )

    g_dram = nc.dram_tensor("g", [K, K], f32, kind="ExternalInput")
    w_dram = nc.dram_tensor("w", [K, K], f32, kind="ExternalInput")
    out_dram = nc.dram_tensor(
        "out", [N_CHUNKS, 128, CHUNK_F], f32, kind="ExternalOutput"
    )

    n_double = (CHUNK_F // K).bit_length() - 1  # doublings from K to CHUNK_F
    assert K * (1 << n_double) == CHUNK_F

    with ExitStack() as ctx:
        ec = ctx.enter_context
        gt = ec(nc.sbuf_tensor("gt", [K, K], f32))
        wt = ec(nc.sbuf_tensor("wt", [K, K], f32))
        ones_col = ec(nc.sbuf_tensor("ones_col", [K, 1], f32))
        ones_row = ec(nc.sbuf_tensor("ones_row", [1, K], f32))
        u_sb = ec(nc.sbuf_tensor("u_sb", [K, 1], f32))
        s_sb = ec(nc.sbuf_tensor("s_sb", [1, K], f32))
        rep = ec(nc.sbuf_tensor("rep", [128, CHUNK_F], f32))
        u_ps = ec(nc.psum_tensor("u_ps", [K, 1], f32))
        s_ps = ec(nc.psum_tensor("s_ps", [1, K], f32))
        b_ps = ec(nc.psum_tensor("b_ps", [128, K], f32))
        sem_g = ec(nc.semaphore("sem_g"))
        sem_w = ec(nc.semaphore("sem_w"))
        sem_p = ec(nc.semaphore("sem_p"))
        sem_v = ec(nc.semaphore("sem_v"))
        sem_out = ec(nc.semaphore("sem_out"))
        block = ec(nc.Block())

        rep_ready = 4 + n_double
        all_stores = 16 * N_CHUNKS

        @block.sync
        def _(sync):
            sync.dma_start(gt[:], g_dram[:]).then_inc(sem_g, 16)
            sync.wait_ge(sem_v, rep_ready)
            for c in range(0, N_CHUNKS, 2):
                sync.dma_start(out_dram[c], rep[:]).then_inc(sem_out, 16)
            sync.wait_ge(sem_out, all_stores)

        @block.scalar
        def _(scalar):
            scalar.dma_start(wt[:], w_dram[:]).then_inc(sem_w, 16)
            scalar.wait_ge(sem_v, rep_ready)
            for c in range(1, N_CHUNKS, 2):
                scalar.dma_start(out_dram[c], rep[:]).then_inc(sem_out, 16)
            scalar.wait_ge(sem_out, all_stores)

        @block.tensor
        def _(tensor):
            tensor.wait_ge(sem_v, 1)
            tensor.wait_ge(sem_g, 16)
            tensor.matmul(
                u_ps[:], gt[:], ones_col[:], start=True, stop=True
            ).then_inc(sem_p, 1)
            tensor.wait_ge(sem_v, 2)
            tensor.wait_ge(sem_w, 16)
            tensor.matmul(
                s_ps[:], u_sb[:], wt[:], start=True, stop=True
            ).then_inc(sem_p, 1)
            tensor.wait_ge(sem_v, 3)
            tensor.matmul(
                b_ps[:], ones_row[:], s_sb[:], start=True, stop=True
            ).then_inc(sem_p, 1)

        @block.vector
        def _(vector):
            vector.memset(ones_col[:], 1.0)
            vector.memset(ones_row[:], 1.0).then_inc(sem_v, 1)
            vector.wait_ge(sem_p, 1)
            vector.tensor_copy(u_sb[:], u_ps[:]).then_inc(sem_v, 1)
            vector.wait_ge(sem_p, 2)
            vector.tensor_copy(s_sb[:], s_ps[:]).then_inc(sem_v, 1)
            vector.wait_ge(sem_p, 3)
            # each doubling reads what the previous DVE op wrote: the DVE
            # pipeline gives no intra-engine RAW guarantee, so chain sems
            vector.tensor_copy(rep[:, 0:K], b_ps[:]).then_inc(sem_v, 1)
            w_cur = K
            for i in range(n_double):
                vector.wait_ge(sem_v, 4 + i)
                vector.tensor_copy(
                    rep[:, w_cur : 2 * w_cur], rep[:, 0:w_cur]
                ).then_inc(sem_v, 1)
                w_cur *= 2

    nc.compile()
    return nc


def _build():
    import concourse.bacc as bacc
    import concourse.mybir as mybir
    import concourse.tile as tile

    f32 = mybir.dt.float32

    nc = bacc.Bacc(
        "TRN2", target_bir_lowering=False, debug=False, num_devices=N_CORES
    )

    g_dram = nc.dram_tensor("g", [K, K], f32, kind="ExternalInput")
    w_dram = nc.dram_tensor("w", [K, K], f32, kind="ExternalInput")
    out_dram = nc.dram_tensor(
        "out", [N_CHUNKS, 128, CHUNK_F], f32, kind="ExternalOutput"
    )

    with tile.TileContext(nc) as tc:
        with (
            tc.tile_pool(name="sbuf", bufs=1) as pool,
            tc.tile_pool(name="psum", bufs=1, space="PSUM") as psum,
        ):
            gt = pool.tile([K, K], f32)
            wt = pool.tile([K, K], f32)
            nc.sync.dma_start(gt[:], g_dram[:])
            nc.scalar.dma_start(wt[:], w_dram[:])

            ones_col = pool.tile([K, 1], f32)
            nc.vector.memset(ones_col[:], 1.0)
            ones_row = pool.tile([1, K], f32)
            nc.vector.memset(ones_row[:], 1.0)

            # u[c] = sum_k g[k, c]   (contract over the k partitions)
            u_ps = psum.tile([K, 1], f32)
            nc.tensor.matmul(u_ps[:], gt[:], ones_col[:], start=True, stop=True)
            u_sb = pool.tile([K, 1], f32)
            nc.vector.tensor_copy(u_sb[:], u_ps[:])

            # s[d] = sum_c u[c] * W[c, d]
            s_ps = psum.tile([1, K], f32)
            nc.tensor.matmul(s_ps[:], u_sb[:], wt[:], start=True, stop=True)
            s_sb = pool.tile([1, K], f32)
            nc.vector.tensor_copy(s_sb[:], s_ps[:])

            # outer product ones(128,1) @ s(1,128): every partition = s
            b_ps = psum.tile([128, K], f32)
            nc.tensor.matmul(b_ps[:], ones_row[:], s_sb[:], start=True, stop=True)

            # replicate along the free dim: 128 -> CHUNK_F floats/partition
            rep = pool.tile([128, CHUNK_F], f32)
            nc.vector.tensor_copy(rep[:, 0:K], b_ps[:])
            w_cur = K
            while w_cur < CHUNK_F:
                nc.vector.tensor_copy(rep[:, w_cur : 2 * w_cur], rep[:, 0:w_cur])
                w_cur *= 2

            # stream the shard out; alternate the two HWDGE issue engines
            for c in range(N_CHUNKS):
                eng = nc.sync if c % 2 == 0 else nc.scalar
                eng.dma_start(out_dram[c], rep[:])

    nc.compile()
    return nc


def _get_nc():
    global _NC
    if _NC is None:
        _NC = _build_raw() if USE_RAW else _build()
    return _NC


def _run(g: np.ndarray, w: np.ndarray, trace: bool = False):
    from concourse.bass_utils import run_bass_kernel_spmd

    nc = _get_nc()
    in_maps = [{"g": g, "w": w} for _ in range(N_CORES)]
    return run_bass_kernel_spmd(nc, in_maps, list(range(N_CORES)), trace=trace)


def kernel(x: np.ndarray, W: np.ndarray, station_ids: np.ndarray) -> np.ndarray:
    x = np.asarray(x, dtype=np.float32)
    W = np.ascontiguousarray(np.asarray(W, dtype=np.float32))
    sid = np.asarray(station_ids).astype(np.int64)

    sx = sid // H
    sy = sid % WD
    g = np.ascontiguousarray(x[sx, sy])  # (K, K) replicated station rows

    res = _run(g, W).results
    shards = [res[c]["out"].reshape(ROWS_PER_CORE, WD, K) for c in range(N_CORES)]
    return np.concatenate(shards, axis=0)
